# revision 1
# baseline (speedup 1.0000x reference)
"""Trainium2 Bass kernel for the 6-level hierarchical Choquet integral tree.

Tree: 16-ary, depth 6, 16.7M leaves. Each node computes a 2-additive Choquet
integral of its 16 children: softmax(theta) over 136 coeffs (16 singles +
120 pair-mins), dot with [children ; pairwise mins].

Sharding: 8 cores, each owns a contiguous subtree of 2M leaves and computes
levels 1-4 on device (output: 32 level-4 values per core). Host finishes the
tiny levels 5-6 (17 nodes) in numpy.

Device layout ("layout A"): nodes on partitions, G nodes per partition row,
features along the free dim. Pair mins are computed with 15 shifted
tensor_tensor(min) ops over the 16 children (pairs grouped by distance d);
theta columns are pre-permuted on the host into the matching d-major order
(softmax and the weighted sum are permutation invariant, so this is exact).
"""

import os

import numpy as np

import concourse.bass as bass
import concourse.mybir as mybir
import concourse.tile as tile
from concourse import bacc
from concourse.bass_utils import run_bass_kernel_spmd

B = 16
NPAIR = 120
NF = B + NPAIR  # 136
NCORE = 8
LEAF_PER_CORE = 16**6 // NCORE  # 2,097,152
# per-core node counts for on-device levels 1..4
LN = [LEAF_PER_CORE // (B**l) for l in range(1, 5)]  # [131072, 8192, 512, 32]

# (P partitions, G nodes per partition row) per level
LEVEL_PG = [(128, 16), (128, 16), (128, 4), (32, 1)]

_F32 = mybir.dt.float32


def _dmajor_perm() -> np.ndarray:
    """Column permutation mapping natural theta order -> [singles, pairs in
    d-major order], where pair (i, i+d) sits at offset off_d + i."""
    II, JJ = np.triu_indices(B, k=1)
    nat = {(int(i), int(j)): p for p, (i, j) in enumerate(zip(II, JJ))}
    perm = list(range(B))
    for d in range(1, B):
        for i in range(B - d):
            perm.append(B + nat[(i, i + d)])
    assert len(perm) == NF
    return np.array(perm, dtype=np.int64)


PERM = _dmajor_perm()
# off_d: start of distance-d block within the 120 pair columns
OFFD = np.concatenate([[0], np.cumsum([B - d for d in range(1, B)])])


def _build_program() -> bass.Bass:
    nc = bacc.Bacc("TRN2", target_bir_lowering=False, debug=False)

    x_d = nc.dram_tensor("x", [LEAF_PER_CORE], _F32, kind="ExternalInput")
    th_d = [
        nc.dram_tensor(f"t{l + 1}", [LN[l] * NF], _F32, kind="ExternalInput")
        for l in range(4)
    ]
    v_d = [nc.dram_tensor(f"v{l + 1}", [LN[l]], _F32) for l in range(3)]
    o4 = nc.dram_tensor("o4", [LN[3]], _F32, kind="ExternalOutput")

    srcs = [x_d.ap()] + [v.ap() for v in v_d]
    dsts = [v.ap() for v in v_d] + [o4.ap()]
    th_aps = [t.ap() for t in th_d]

    with tile.TileContext(nc) as tc:
        _kernel_body(nc, tc, th_aps, srcs, dsts, LN, LEVEL_PG)
    nc.compile()
    return nc


def _kernel_body(nc, tc, th_aps, srcs, dsts, lns, level_pg) -> None:
    if True:
        with (
            tc.tile_pool(name="th", bufs=3) as thp,
            tc.tile_pool(name="xs", bufs=3) as xsp,
            tc.tile_pool(name="e", bufs=2) as ep,
            tc.tile_pool(name="buf", bufs=2) as bp,
            tc.tile_pool(name="small", bufs=3) as sp,
        ):
            for lvl in range(len(lns)):
                P, G = level_pg[lvl]
                n_nodes = lns[lvl]
                ntile = P * G
                T = n_nodes // ntile
                th_src = th_aps[lvl].rearrange(
                    "(t p f) -> t p f", p=P, f=G * NF
                )
                xs_src = srcs[lvl].rearrange("(t p q) -> t p q", p=P, q=G * B)
                out_dst = dsts[lvl].rearrange("(t p g) -> t p g", p=P, g=G)

                for t in range(T):
                    th_t = thp.tile([P, G * NF], _F32, tag="th")
                    nc.gpsimd.dma_start(out=th_t[:], in_=th_src[t])
                    xs_t = xsp.tile([P, G * B], _F32, tag="xs")
                    nc.gpsimd.dma_start(out=xs_t[:], in_=xs_src[t])

                    e_t = ep.tile([P, G * NF], _F32, tag="e")
                    nc.scalar.activation(
                        e_t[:], th_t[:], mybir.ActivationFunctionType.Exp
                    )

                    buf_t = bp.tile([P, G * NF], _F32, tag="buf")
                    b3 = buf_t[:].rearrange("p (g f) -> p g f", g=G)
                    x3 = xs_t[:].rearrange("p (g f) -> p g f", g=G)
                    # pair mins, distance-major
                    for d in range(1, B):
                        c = B - d
                        o = B + int(OFFD[d - 1])
                        nc.vector.tensor_tensor(
                            b3[:, :, o : o + c],
                            x3[:, :, 0:c],
                            x3[:, :, d:B],
                            op=mybir.AluOpType.min,
                        )

                    e3 = e_t[:].rearrange("p (g f) -> p g f", g=G)
                    den_t = sp.tile([P, G], _F32, tag="den")
                    nc.vector.tensor_reduce(
                        den_t[:], e3, axis=mybir.AxisListType.X, op=mybir.AluOpType.add
                    )
                    # prod: singles e*xs land in buf[:,:, :16]; pairs in place
                    nc.vector.tensor_tensor(
                        b3[:, :, 0:B], e3[:, :, 0:B], x3[:, :, :],
                        op=mybir.AluOpType.mult,
                    )
                    nc.vector.tensor_tensor(
                        b3[:, :, B:], b3[:, :, B:], e3[:, :, B:],
                        op=mybir.AluOpType.mult,
                    )
                    dot_t = sp.tile([P, G], _F32, tag="dot")
                    nc.vector.tensor_reduce(
                        dot_t[:], b3, axis=mybir.AxisListType.X, op=mybir.AluOpType.add
                    )
                    rcp_t = sp.tile([P, G], _F32, tag="rcp")
                    nc.vector.reciprocal(rcp_t[:], den_t[:])
                    out_t = sp.tile([P, G], _F32, tag="out")
                    nc.vector.tensor_mul(out_t[:], dot_t[:], rcp_t[:])
                    nc.gpsimd.dma_start(out=out_dst[t], in_=out_t[:])


_PROG_CACHE: bass.Bass | None = None
LAST_RESULTS = None  # BassKernelResults of the most recent kernel() call


def _ensure_ntff_hook() -> None:
    """Provide antenv.axon_hooks + the ctypes NTFF hook when the image
    lacks them, so trace=True produces a perfetto profile under axon."""
    import contextlib
    import ctypes
    import sys
    import types

    try:
        from antenv.axon_hooks import get_axon_ntff_profile_hook  # noqa: F401

        return
    except ImportError:
        pass

    import antenv
    import concourse.bass_utils as bu

    holder = {"h": None}
    mod = types.ModuleType("antenv.axon_hooks")
    mod.set_axon_ntff_profile_hook = lambda h: holder.__setitem__("h", h)
    mod.get_axon_ntff_profile_hook = lambda: holder["h"]
    sys.modules["antenv.axon_hooks"] = mod
    antenv.axon_hooks = mod
    bu.upload_artifacts = lambda tmpdir: ""  # no artifact bucket here

    so_path = "/opt/axon/libaxon_pjrt.so"
    try:
        lib = ctypes.CDLL(so_path)
    except OSError:
        return
    if not hasattr(lib, "axon_start_nrt_profile"):
        return
    lib.axon_start_nrt_profile.argtypes = [
        ctypes.POINTER(ctypes.c_int64),
        ctypes.c_size_t,
    ]
    lib.axon_start_nrt_profile.restype = ctypes.c_int64
    lib.axon_stop_nrt_profile.argtypes = [ctypes.c_char_p]
    lib.axon_stop_nrt_profile.restype = ctypes.c_int64

    @contextlib.contextmanager
    def _hook(output_dir, device_ids):
        import jax

        jax.devices()
        if device_ids:
            ids = (ctypes.c_int64 * len(device_ids))(*device_ids)
            rc = lib.axon_start_nrt_profile(ids, len(device_ids))
        else:
            rc = lib.axon_start_nrt_profile(None, 0)
        if rc != 0:
            raise RuntimeError(f"axon_start_nrt_profile rc={rc}")
        try:
            yield
        finally:
            n = lib.axon_stop_nrt_profile(str(output_dir).encode())
            print(f"profile: {n} file(s) written to {output_dir}")

    mod.set_axon_ntff_profile_hook(_hook)


def _choquet_np(vals: np.ndarray, theta: np.ndarray) -> np.ndarray:
    II, JJ = np.triu_indices(B, k=1)
    n = theta.shape[0]
    xs = vals.reshape(n, B).astype(np.float64)
    t = theta.astype(np.float64)
    e = np.exp(t - t.max(axis=1, keepdims=True))
    m = e / e.sum(axis=1, keepdims=True)
    mins = np.minimum(xs[:, II], xs[:, JJ])
    return (m[:, :B] * xs).sum(axis=1) + (m[:, B:] * mins).sum(axis=1)


def kernel(x, theta1, theta2, theta3, theta4, theta5, theta6) -> np.ndarray:
    global _PROG_CACHE, LAST_RESULTS
    x = np.ascontiguousarray(np.asarray(x, dtype=np.float32).reshape(-1))
    ths = []
    for th in (theta1, theta2, theta3, theta4):
        th = np.asarray(th, dtype=np.float32)
        ths.append(np.ascontiguousarray(th[:, PERM]))

    if _PROG_CACHE is None:
        _PROG_CACHE = _build_program()
    nc = _PROG_CACHE

    in_maps = []
    for c in range(NCORE):
        m = {"x": x[c * LEAF_PER_CORE : (c + 1) * LEAF_PER_CORE]}
        for l in range(4):
            rows = LN[l]
            m[f"t{l + 1}"] = ths[l][c * rows : (c + 1) * rows].reshape(-1)
        in_maps.append(m)

    trace = os.environ.get("BASS_KERNEL_TRACE", "0") == "1"
    if trace:
        _ensure_ntff_hook()
    res = run_bass_kernel_spmd(nc, in_maps, list(range(NCORE)), trace=trace)
    LAST_RESULTS = res

    l4 = np.concatenate([res.results[c]["o4"].reshape(-1) for c in range(NCORE)])
    l5 = _choquet_np(l4, np.asarray(theta5, dtype=np.float32))
    l6 = _choquet_np(l5, np.asarray(theta6, dtype=np.float32))
    return l6.astype(np.float32).reshape((1,))



# revision 2
# speedup vs baseline: 1.2966x; 1.2966x over previous
"""Trainium2 Bass kernel for the 6-level hierarchical Choquet integral tree.

Tree: 16-ary, depth 6, 16.7M leaves. Each node computes a 2-additive Choquet
integral of its 16 children: softmax(theta) over 136 coeffs (16 singles +
120 pair-mins), dot with [children ; pairwise mins].

Sharding: 8 cores, each owns a contiguous subtree of 2M leaves and computes
levels 1-4 on device (output: 32 level-4 values per core). Host finishes the
tiny levels 5-6 (17 nodes) in numpy.

Device layout ("layout A"): nodes on partitions, G nodes per partition row,
features along the free dim, all bf16 (DVE tensor_tensor runs 2x_1p on
bf16; HBM traffic halves). Per tile:
  - ScalarE: exp(theta)               (1x, dtype-independent)
  - DVE: 15 shifted tensor_tensor(min) ops (d-major pair order), 2 product
    ops, then a halving add-tree 136->68->34->17 + tensor_reduce(17) for the
    numerator (the tree runs at 2x where tensor_reduce is stuck at 1x)
  - GPSIMD: the same halving tree for the softmax denominator, off the
    critical DVE path
  - DVE: reciprocal + multiply -> node outputs
"""

import os

import numpy as np

import concourse.bass as bass
import concourse.mybir as mybir
import concourse.tile as tile
from concourse import bacc
from concourse.bass_utils import run_bass_kernel_spmd

B = 16
NPAIR = 120
NF = B + NPAIR  # 136
NCORE = 8
LEAF_PER_CORE = 16**6 // NCORE  # 2,097,152
# per-core node counts for on-device levels 1..4
LN = [LEAF_PER_CORE // (B**l) for l in range(1, 5)]  # [131072, 8192, 512, 32]

# (P partitions, G nodes per partition row) per level
LEVEL_PG = [(128, 64), (128, 64), (128, 4), (32, 1)]

_F32 = mybir.dt.float32
_BF16 = mybir.dt.bfloat16


def _dmajor_perm() -> np.ndarray:
    """Column permutation mapping natural theta order -> [singles, pairs in
    d-major order], where pair (i, i+d) sits at offset off_d + i."""
    II, JJ = np.triu_indices(B, k=1)
    nat = {(int(i), int(j)): p for p, (i, j) in enumerate(zip(II, JJ))}
    perm = list(range(B))
    for d in range(1, B):
        for i in range(B - d):
            perm.append(B + nat[(i, i + d)])
    assert len(perm) == NF
    return np.array(perm, dtype=np.int64)


PERM = _dmajor_perm()
# off_d: start of distance-d block within the 120 pair columns
OFFD = np.concatenate([[0], np.cumsum([B - d for d in range(1, B)])])


def _build_program() -> bass.Bass:
    nc = bacc.Bacc("TRN2", target_bir_lowering=False, debug=False)

    x_d = nc.dram_tensor("x", [LEAF_PER_CORE], _BF16, kind="ExternalInput")
    th_d = [
        nc.dram_tensor(f"t{l + 1}", [LN[l] * NF], _BF16, kind="ExternalInput")
        for l in range(4)
    ]
    v_d = [nc.dram_tensor(f"v{l + 1}", [LN[l]], _BF16) for l in range(3)]
    o4 = nc.dram_tensor("o4", [LN[3]], _F32, kind="ExternalOutput")

    srcs = [x_d.ap()] + [v.ap() for v in v_d]
    dsts = [v.ap() for v in v_d] + [o4.ap()]
    th_aps = [t.ap() for t in th_d]

    with tile.TileContext(nc) as tc:
        _kernel_body(nc, tc, th_aps, srcs, dsts, LN, LEVEL_PG)
    nc.compile()
    return nc


def _kernel_body(nc, tc, th_aps, srcs, dsts, lns, level_pg) -> None:
    with (
        tc.tile_pool(name="th", bufs=2) as thp,
        tc.tile_pool(name="xs", bufs=2) as xsp,
        tc.tile_pool(name="e", bufs=2) as ep,
        tc.tile_pool(name="m", bufs=1) as mp,
        tc.tile_pool(name="p", bufs=1) as pp,
        tc.tile_pool(name="t", bufs=1) as tp,
        tc.tile_pool(name="u", bufs=2) as up,
        tc.tile_pool(name="small", bufs=2) as sp,
    ):
        for lvl in range(len(lns)):
            P, G = level_pg[lvl]
            n_nodes = lns[lvl]
            ntile = P * G
            T = n_nodes // ntile
            assert T * ntile == n_nodes
            th_src = th_aps[lvl].rearrange("(t p f) -> t p f", p=P, f=G * NF)
            xs_src = srcs[lvl].rearrange("(t p q) -> t p q", p=P, q=G * B)
            out_dst = dsts[lvl].rearrange("(t p g) -> t p g", p=P, g=G)
            out_dt = _F32 if lvl == 3 else _BF16

            for t in range(T):
                th_t = thp.tile([P, G * NF], _BF16, tag="th")
                nc.sync.dma_start(out=th_t[:], in_=th_src[t])
                xs_t = xsp.tile([P, G * B], _BF16, tag="xs")
                nc.sync.dma_start(out=xs_t[:], in_=xs_src[t])

                e_t = ep.tile([P, G * NF], _BF16, tag="e")
                nc.scalar.activation(
                    e_t[:], th_t[:], mybir.ActivationFunctionType.Exp
                )
                e3 = e_t[:].rearrange("p (g f) -> p g f", g=G)

                # pair mins, distance-major: m3[:, :, off_d + i] = min(x_i, x_{i+d})
                m_t = mp.tile([P, G * NPAIR], _BF16, tag="m")
                m3 = m_t[:].rearrange("p (g f) -> p g f", g=G)
                x3 = xs_t[:].rearrange("p (g f) -> p g f", g=G)
                for d in range(1, B):
                    c = B - d
                    o = int(OFFD[d - 1])
                    nc.vector.tensor_tensor(
                        m3[:, :, o : o + c],
                        x3[:, :, 0:c],
                        x3[:, :, d:B],
                        op=mybir.AluOpType.min,
                    )

                # products: p3[:, :, :16] = e_s * x ; p3[:, :, 16:] = e_p * mins
                p_t = pp.tile([P, G * NF], _BF16, tag="p")
                p3 = p_t[:].rearrange("p (g f) -> p g f", g=G)
                nc.vector.tensor_tensor(
                    p3[:, :, 0:B], e3[:, :, 0:B], x3, op=mybir.AluOpType.mult
                )
                nc.vector.tensor_tensor(
                    p3[:, :, B:NF], e3[:, :, B:NF], m3, op=mybir.AluOpType.mult
                )

                # numerator tree on DVE: 136 -> 68 -> 34 -> 17 -> reduce
                t1 = tp.tile([P, G * 68], _BF16, tag="t1")
                t13 = t1[:].rearrange("p (g f) -> p g f", g=G)
                nc.vector.tensor_tensor(
                    t13, p3[:, :, 0:68], p3[:, :, 68:136], op=mybir.AluOpType.add
                )
                t2 = tp.tile([P, G * 34], _BF16, tag="t2")
                t23 = t2[:].rearrange("p (g f) -> p g f", g=G)
                nc.vector.tensor_tensor(
                    t23, t13[:, :, 0:34], t13[:, :, 34:68], op=mybir.AluOpType.add
                )
                t3 = tp.tile([P, G * 17], _BF16, tag="t3")
                t33 = t3[:].rearrange("p (g f) -> p g f", g=G)
                nc.vector.tensor_tensor(
                    t33, t23[:, :, 0:17], t23[:, :, 17:34], op=mybir.AluOpType.add
                )
                num_t = sp.tile([P, G], _F32, tag="num")
                nc.vector.tensor_reduce(
                    num_t[:], t33, axis=mybir.AxisListType.X, op=mybir.AluOpType.add
                )

                # denominator tree on GPSIMD (parallel with DVE work above)
                u1 = up.tile([P, G * 68], _BF16, tag="u1")
                u13 = u1[:].rearrange("p (g f) -> p g f", g=G)
                nc.gpsimd.tensor_tensor(
                    u13, e3[:, :, 0:68], e3[:, :, 68:136], op=mybir.AluOpType.add
                )
                u2 = up.tile([P, G * 34], _BF16, tag="u2")
                u23 = u2[:].rearrange("p (g f) -> p g f", g=G)
                nc.gpsimd.tensor_tensor(
                    u23, u13[:, :, 0:34], u13[:, :, 34:68], op=mybir.AluOpType.add
                )
                u3 = up.tile([P, G * 17], _BF16, tag="u3")
                u33 = u3[:].rearrange("p (g f) -> p g f", g=G)
                nc.gpsimd.tensor_tensor(
                    u33, u23[:, :, 0:17], u23[:, :, 17:34], op=mybir.AluOpType.add
                )
                den_t = sp.tile([P, G], _F32, tag="den")
                nc.vector.tensor_reduce(
                    den_t[:], u33, axis=mybir.AxisListType.X, op=mybir.AluOpType.add
                )

                rcp_t = sp.tile([P, G], _F32, tag="rcp")
                nc.vector.reciprocal(rcp_t[:], den_t[:])
                out_t = sp.tile([P, G], out_dt, tag="out")
                nc.vector.tensor_mul(out_t[:], num_t[:], rcp_t[:])
                nc.sync.dma_start(out=out_dst[t], in_=out_t[:])


_PROG_CACHE: bass.Bass | None = None
LAST_RESULTS = None  # BassKernelResults of the most recent kernel() call


def _ensure_ntff_hook() -> None:
    """Provide antenv.axon_hooks + the ctypes NTFF hook when the image
    lacks them, so trace=True produces a perfetto profile under axon."""
    import contextlib
    import ctypes
    import sys
    import types

    try:
        from antenv.axon_hooks import get_axon_ntff_profile_hook  # noqa: F401

        return
    except ImportError:
        pass

    import antenv
    import concourse.bass_utils as bu

    holder = {"h": None}
    mod = types.ModuleType("antenv.axon_hooks")
    mod.set_axon_ntff_profile_hook = lambda h: holder.__setitem__("h", h)
    mod.get_axon_ntff_profile_hook = lambda: holder["h"]
    sys.modules["antenv.axon_hooks"] = mod
    antenv.axon_hooks = mod
    bu.upload_artifacts = lambda tmpdir: ""  # no artifact bucket here

    so_path = "/opt/axon/libaxon_pjrt.so"
    try:
        lib = ctypes.CDLL(so_path)
    except OSError:
        return
    if not hasattr(lib, "axon_start_nrt_profile"):
        return
    lib.axon_start_nrt_profile.argtypes = [
        ctypes.POINTER(ctypes.c_int64),
        ctypes.c_size_t,
    ]
    lib.axon_start_nrt_profile.restype = ctypes.c_int64
    lib.axon_stop_nrt_profile.argtypes = [ctypes.c_char_p]
    lib.axon_stop_nrt_profile.restype = ctypes.c_int64

    @contextlib.contextmanager
    def _hook(output_dir, device_ids):
        import jax

        jax.devices()
        if device_ids:
            ids = (ctypes.c_int64 * len(device_ids))(*device_ids)
            rc = lib.axon_start_nrt_profile(ids, len(device_ids))
        else:
            rc = lib.axon_start_nrt_profile(None, 0)
        if rc != 0:
            raise RuntimeError(f"axon_start_nrt_profile rc={rc}")
        try:
            yield
        finally:
            n = lib.axon_stop_nrt_profile(str(output_dir).encode())
            print(f"profile: {n} file(s) written to {output_dir}")

    mod.set_axon_ntff_profile_hook(_hook)


def _choquet_np(vals: np.ndarray, theta: np.ndarray) -> np.ndarray:
    II, JJ = np.triu_indices(B, k=1)
    n = theta.shape[0]
    xs = vals.reshape(n, B).astype(np.float64)
    t = theta.astype(np.float64)
    e = np.exp(t - t.max(axis=1, keepdims=True))
    m = e / e.sum(axis=1, keepdims=True)
    mins = np.minimum(xs[:, II], xs[:, JJ])
    return (m[:, :B] * xs).sum(axis=1) + (m[:, B:] * mins).sum(axis=1)


def kernel(x, theta1, theta2, theta3, theta4, theta5, theta6) -> np.ndarray:
    global _PROG_CACHE, LAST_RESULTS
    import ml_dtypes

    bf16 = ml_dtypes.bfloat16
    x = np.ascontiguousarray(np.asarray(x, dtype=np.float32).reshape(-1)).astype(bf16)
    ths = []
    for th in (theta1, theta2, theta3, theta4):
        th = np.asarray(th, dtype=np.float32)
        ths.append(np.ascontiguousarray(th[:, PERM].astype(bf16)))

    if _PROG_CACHE is None:
        _PROG_CACHE = _build_program()
    nc = _PROG_CACHE

    in_maps = []
    for c in range(NCORE):
        m = {"x": x[c * LEAF_PER_CORE : (c + 1) * LEAF_PER_CORE]}
        for l in range(4):
            rows = LN[l]
            m[f"t{l + 1}"] = ths[l][c * rows : (c + 1) * rows].reshape(-1)
        in_maps.append(m)

    trace = os.environ.get("BASS_KERNEL_TRACE", "0") == "1"
    if trace:
        _ensure_ntff_hook()
    res = run_bass_kernel_spmd(nc, in_maps, list(range(NCORE)), trace=trace)
    LAST_RESULTS = res

    l4 = np.concatenate(
        [np.asarray(res.results[c]["o4"], dtype=np.float32).reshape(-1) for c in range(NCORE)]
    )
    l5 = _choquet_np(l4, np.asarray(theta5, dtype=np.float32))
    l6 = _choquet_np(l5, np.asarray(theta6, dtype=np.float32))
    return l6.astype(np.float32).reshape((1,))


# revision 4
# speedup vs baseline: 1.6296x; 1.2568x over previous
"""Trainium2 Bass kernel for the 6-level hierarchical Choquet integral tree.

Tree: 16-ary, depth 6, 16.7M leaves. Each node computes a 2-additive Choquet
integral of its 16 children: softmax(theta) over 136 coeffs (16 singles +
120 pair-mins), dot with [children ; pairwise mins].

Sharding: 8 cores, each owns a contiguous subtree of 2M leaves and computes
levels 1-4 on device (output: 32 level-4 values per core). Host finishes the
tiny levels 5-6 (17 nodes) in numpy.

Device layout ("layout A"): nodes on partitions, G nodes per partition row,
features along the free dim, all bf16 (DVE tensor_tensor runs 2x_1p on
bf16; HBM traffic halves). Per tile:
  - ScalarE: exp(theta)               (1x, dtype-independent)
  - DVE: 15 shifted tensor_tensor(min) ops (d-major pair order), 2 product
    ops, then a halving add-tree 136->68->34->17 + tensor_reduce(17) for the
    numerator (the tree runs at 2x where tensor_reduce is stuck at 1x)
  - GPSIMD: the same halving tree for the softmax denominator, off the
    critical DVE path
  - DVE: reciprocal + multiply -> node outputs
"""

import os

import numpy as np

import concourse.bass as bass
import concourse.mybir as mybir
import concourse.tile as tile
from concourse import bacc
from concourse.bass_utils import run_bass_kernel_spmd
from concourse.masks import make_identity

B = 16
NPAIR = 120
NF = B + NPAIR  # 136
NCORE = 8
LEAF_PER_CORE = 16**6 // NCORE  # 2,097,152
# per-core node counts for on-device levels 1..4
LN = [LEAF_PER_CORE // (B**l) for l in range(1, 5)]  # [131072, 8192, 512, 32]

# (P partitions, G nodes per partition row) per level
LEVEL_PG = [(128, 64), (128, 64), (128, 4), (32, 1)]

_F32 = mybir.dt.float32
_BF16 = mybir.dt.bfloat16


def _dmajor_perm() -> np.ndarray:
    """Column permutation mapping natural theta order -> [singles, pairs in
    d-major order], where pair (i, i+d) sits at offset off_d + i."""
    II, JJ = np.triu_indices(B, k=1)
    nat = {(int(i), int(j)): p for p, (i, j) in enumerate(zip(II, JJ))}
    perm = list(range(B))
    for d in range(1, B):
        for i in range(B - d):
            perm.append(B + nat[(i, i + d)])
    assert len(perm) == NF
    return np.array(perm, dtype=np.int64)


PERM = _dmajor_perm()
# off_d: start of distance-d block within the 120 pair columns
OFFD = np.concatenate([[0], np.cumsum([B - d for d in range(1, B)])])


def _build_program() -> bass.Bass:
    nc = bacc.Bacc("TRN2", target_bir_lowering=False, debug=False)

    x_d = nc.dram_tensor("x", [LEAF_PER_CORE], _BF16, kind="ExternalInput")
    th_d = [
        nc.dram_tensor(f"t{l + 1}", [LN[l] * NF], _BF16, kind="ExternalInput")
        for l in range(4)
    ]
    v_d = [nc.dram_tensor(f"v{l + 1}", [LN[l]], _BF16) for l in range(3)]
    o4 = nc.dram_tensor("o4", [LN[3]], _F32, kind="ExternalOutput")

    srcs = [x_d.ap()] + [v.ap() for v in v_d]
    dsts = [v.ap() for v in v_d] + [o4.ap()]
    th_aps = [t.ap() for t in th_d]

    with tile.TileContext(nc) as tc:
        _kernel_body(nc, tc, th_aps, srcs, dsts, LN, LEVEL_PG)
    nc.compile()
    return nc


def _kernel_body(nc, tc, th_aps, srcs, dsts, lns, level_pg) -> None:
    with (
        tc.tile_pool(name="consts", bufs=1) as cp,
        tc.tile_pool(name="th", bufs=2) as thp,
        tc.tile_pool(name="xs", bufs=2) as xsp,
        tc.tile_pool(name="e", bufs=2) as ep,
        tc.tile_pool(name="m", bufs=2) as mp,
        tc.tile_pool(name="pp", bufs=2) as ppp,
        tc.tile_pool(name="ps", bufs=2) as psp,
        tc.tile_pool(name="acc", bufs=2, space="PSUM") as accp,
        tc.tile_pool(name="small", bufs=2) as sp,
    ):
        ident = cp.tile([128, 128], _BF16, tag="id")
        make_identity(nc, ident[:])

        for lvl in range(len(lns)):
            P, G = level_pg[lvl]
            n_nodes = lns[lvl]
            ntile = P * G
            T = n_nodes // ntile
            assert T * ntile == n_nodes
            th_src = th_aps[lvl].rearrange("(t p f) -> t p f", p=P, f=G * NF)
            xs_src = srcs[lvl].rearrange("(t p q) -> t p q", p=P, q=G * B)
            out_dst = dsts[lvl].rearrange("(t p g) -> t p g", p=P, g=G)
            out_dt = _F32 if lvl == 3 else _BF16
            idp = ident[:][0:P, 0:P]

            for t in range(T):
                th_t = thp.tile([P, G * NF], _BF16, tag="th")
                nc.sync.dma_start(out=th_t[:], in_=th_src[t])
                xs_t = xsp.tile([P, G * B], _BF16, tag="xs")
                nc.sync.dma_start(out=xs_t[:], in_=xs_src[t])

                e_t = ep.tile([P, G * NF], _BF16, tag="e")
                nc.scalar.activation(
                    e_t[:], th_t[:], mybir.ActivationFunctionType.Exp
                )
                e3 = e_t[:].rearrange("p (g f) -> p g f", g=G)

                # pair mins, distance-major: m3[:, :, off_d + i] = min(x_i, x_{i+d})
                m_t = mp.tile([P, G * NPAIR], _BF16, tag="m")
                m3 = m_t[:].rearrange("p (g f) -> p g f", g=G)
                x3 = xs_t[:].rearrange("p (g f) -> p g f", g=G)
                for d in range(1, B):
                    c = B - d
                    o = int(OFFD[d - 1])
                    nc.vector.tensor_tensor(
                        m3[:, :, o : o + c],
                        x3[:, :, 0:c],
                        x3[:, :, d:B],
                        op=mybir.AluOpType.min,
                    )

                # products: singles on DVE, pairs on GPSIMD (runs concurrently)
                ps_t = psp.tile([P, G * B], _BF16, tag="ps")
                nc.vector.tensor_tensor(
                    ps_t[:].rearrange("p (g f) -> p g f", g=G),
                    e3[:, :, 0:B],
                    x3,
                    op=mybir.AluOpType.mult,
                )
                pp_t = ppp.tile([P, G * NPAIR], _BF16, tag="pp")
                nc.gpsimd.tensor_tensor(
                    pp_t[:].rearrange("p (g f) -> p g f", g=G),
                    e3[:, :, B:NF],
                    m3,
                    op=mybir.AluOpType.mult,
                )

                # PE segmented reduction: accumulate 136 features -> 8 per node
                # via identity-stationary matmuls summing strided j-slices.
                accn = accp.tile([P, G * 8], _F32, tag="accn")
                accn3 = accn[:].rearrange("p (g j) -> p g j", g=G)
                ps4 = ps_t[:].rearrange("p (g t j) -> p g t j", g=G, t=2, j=8)
                pp4 = pp_t[:].rearrange("p (g t j) -> p g t j", g=G, t=15, j=8)
                for k in range(2):
                    nc.tensor.matmul(
                        accn3, idp, ps4[:, :, k, :], start=(k == 0), stop=False
                    )
                for k in range(15):
                    nc.tensor.matmul(
                        accn3, idp, pp4[:, :, k, :], start=False, stop=(k == 14)
                    )

                accd = accp.tile([P, G * 8], _F32, tag="accd")
                accd3 = accd[:].rearrange("p (g j) -> p g j", g=G)
                e4 = e_t[:].rearrange("p (g t j) -> p g t j", g=G, t=17, j=8)
                for k in range(17):
                    nc.tensor.matmul(
                        accd3, idp, e4[:, :, k, :], start=(k == 0), stop=(k == 16)
                    )

                # remainder reduce 8 -> 1 (PSUM source), then normalize
                num_t = sp.tile([P, G], _F32, tag="num")
                nc.vector.tensor_reduce(
                    num_t[:], accn3, axis=mybir.AxisListType.X, op=mybir.AluOpType.add
                )
                den_t = sp.tile([P, G], _F32, tag="den")
                nc.vector.tensor_reduce(
                    den_t[:], accd3, axis=mybir.AxisListType.X, op=mybir.AluOpType.add
                )
                rcp_t = sp.tile([P, G], _F32, tag="rcp")
                nc.vector.reciprocal(rcp_t[:], den_t[:])
                out_t = sp.tile([P, G], out_dt, tag="out")
                nc.vector.tensor_mul(out_t[:], num_t[:], rcp_t[:])
                nc.sync.dma_start(out=out_dst[t], in_=out_t[:])


_PROG_CACHE: bass.Bass | None = None
LAST_RESULTS = None  # BassKernelResults of the most recent kernel() call


def _ensure_ntff_hook() -> None:
    """Provide antenv.axon_hooks + the ctypes NTFF hook when the image
    lacks them, so trace=True produces a perfetto profile under axon."""
    import contextlib
    import ctypes
    import sys
    import types

    try:
        from antenv.axon_hooks import get_axon_ntff_profile_hook  # noqa: F401

        return
    except ImportError:
        pass

    import antenv
    import concourse.bass_utils as bu

    holder = {"h": None}
    mod = types.ModuleType("antenv.axon_hooks")
    mod.set_axon_ntff_profile_hook = lambda h: holder.__setitem__("h", h)
    mod.get_axon_ntff_profile_hook = lambda: holder["h"]
    sys.modules["antenv.axon_hooks"] = mod
    antenv.axon_hooks = mod
    bu.upload_artifacts = lambda tmpdir: ""  # no artifact bucket here

    so_path = "/opt/axon/libaxon_pjrt.so"
    try:
        lib = ctypes.CDLL(so_path)
    except OSError:
        return
    if not hasattr(lib, "axon_start_nrt_profile"):
        return
    lib.axon_start_nrt_profile.argtypes = [
        ctypes.POINTER(ctypes.c_int64),
        ctypes.c_size_t,
    ]
    lib.axon_start_nrt_profile.restype = ctypes.c_int64
    lib.axon_stop_nrt_profile.argtypes = [ctypes.c_char_p]
    lib.axon_stop_nrt_profile.restype = ctypes.c_int64

    @contextlib.contextmanager
    def _hook(output_dir, device_ids):
        import jax

        jax.devices()
        if device_ids:
            ids = (ctypes.c_int64 * len(device_ids))(*device_ids)
            rc = lib.axon_start_nrt_profile(ids, len(device_ids))
        else:
            rc = lib.axon_start_nrt_profile(None, 0)
        if rc != 0:
            raise RuntimeError(f"axon_start_nrt_profile rc={rc}")
        try:
            yield
        finally:
            n = lib.axon_stop_nrt_profile(str(output_dir).encode())
            print(f"profile: {n} file(s) written to {output_dir}")

    mod.set_axon_ntff_profile_hook(_hook)


def _choquet_np(vals: np.ndarray, theta: np.ndarray) -> np.ndarray:
    II, JJ = np.triu_indices(B, k=1)
    n = theta.shape[0]
    xs = vals.reshape(n, B).astype(np.float64)
    t = theta.astype(np.float64)
    e = np.exp(t - t.max(axis=1, keepdims=True))
    m = e / e.sum(axis=1, keepdims=True)
    mins = np.minimum(xs[:, II], xs[:, JJ])
    return (m[:, :B] * xs).sum(axis=1) + (m[:, B:] * mins).sum(axis=1)


def kernel(x, theta1, theta2, theta3, theta4, theta5, theta6) -> np.ndarray:
    global _PROG_CACHE, LAST_RESULTS
    import ml_dtypes

    bf16 = ml_dtypes.bfloat16
    x = np.ascontiguousarray(np.asarray(x, dtype=np.float32).reshape(-1)).astype(bf16)
    ths = []
    for th in (theta1, theta2, theta3, theta4):
        th = np.asarray(th, dtype=np.float32)
        ths.append(np.ascontiguousarray(th[:, PERM].astype(bf16)))

    if _PROG_CACHE is None:
        _PROG_CACHE = _build_program()
    nc = _PROG_CACHE

    in_maps = []
    for c in range(NCORE):
        m = {"x": x[c * LEAF_PER_CORE : (c + 1) * LEAF_PER_CORE]}
        for l in range(4):
            rows = LN[l]
            m[f"t{l + 1}"] = ths[l][c * rows : (c + 1) * rows].reshape(-1)
        in_maps.append(m)

    trace = os.environ.get("BASS_KERNEL_TRACE", "0") == "1"
    if trace:
        _ensure_ntff_hook()
    res = run_bass_kernel_spmd(nc, in_maps, list(range(NCORE)), trace=trace)
    LAST_RESULTS = res

    l4 = np.concatenate(
        [np.asarray(res.results[c]["o4"], dtype=np.float32).reshape(-1) for c in range(NCORE)]
    )
    l5 = _choquet_np(l4, np.asarray(theta5, dtype=np.float32))
    l6 = _choquet_np(l5, np.asarray(theta6, dtype=np.float32))
    return l6.astype(np.float32).reshape((1,))


# revision 7
# speedup vs baseline: 2.7306x; 1.6756x over previous
"""Trainium2 Bass kernel for the 6-level hierarchical Choquet integral tree.

Tree: 16-ary, depth 6, 16.7M leaves. Each node computes a 2-additive Choquet
integral of its 16 children: softmax(theta) over 136 coeffs (16 singles +
120 pair-mins), dot with [children ; pairwise mins].

Sharding: 8 cores, each owns a contiguous subtree of 2M leaves and computes
levels 1-4 on device (output: 32 level-4 values per core). Host finishes the
tiny levels 5-6 (17 nodes) in numpy.

Device layout ("layout A"): nodes on partitions, G nodes per partition row,
features along the free dim, all bf16 (DVE tensor_tensor runs 2x_1p on
bf16; HBM traffic halves). Per tile:
  - ScalarE: exp(theta)               (1x, dtype-independent)
  - DVE: 15 shifted tensor_tensor(min) ops (d-major pair order), 2 product
    ops, then a halving add-tree 136->68->34->17 + tensor_reduce(17) for the
    numerator (the tree runs at 2x where tensor_reduce is stuck at 1x)
  - GPSIMD: the same halving tree for the softmax denominator, off the
    critical DVE path
  - DVE: reciprocal + multiply -> node outputs
"""

import os

import numpy as np

import concourse.bass as bass
import concourse.mybir as mybir
import concourse.tile as tile
from concourse import bacc
from concourse.bass_utils import run_bass_kernel_spmd
from concourse.masks import make_identity

B = 16
NPAIR = 120
NF = B + NPAIR  # 136
NCORE = 8
LEAF_PER_CORE = 16**6 // NCORE  # 2,097,152
# per-core node counts for on-device levels 1..4
LN = [LEAF_PER_CORE // (B**l) for l in range(1, 5)]  # [131072, 8192, 512, 32]

# (P partitions, G nodes per partition row) per level
LEVEL_PG = [(128, 64), (128, 64), (128, 4), (32, 1)]

_F32 = mybir.dt.float32
_BF16 = mybir.dt.bfloat16


def _dmajor_perm() -> np.ndarray:
    """Column permutation mapping natural theta order -> [singles, pairs in
    d-major order], where pair (i, i+d) sits at offset off_d + i."""
    II, JJ = np.triu_indices(B, k=1)
    nat = {(int(i), int(j)): p for p, (i, j) in enumerate(zip(II, JJ))}
    perm = list(range(B))
    for d in range(1, B):
        for i in range(B - d):
            perm.append(B + nat[(i, i + d)])
    assert len(perm) == NF
    return np.array(perm, dtype=np.int64)


PERM = _dmajor_perm()
# off_d: start of distance-d block within the 120 pair columns
OFFD = np.concatenate([[0], np.cumsum([B - d for d in range(1, B)])])


def _build_program() -> bass.Bass:
    nc = bacc.Bacc("TRN2", target_bir_lowering=False, debug=False)

    x_d = nc.dram_tensor("x", [LEAF_PER_CORE], _BF16, kind="ExternalInput")
    th_d = [
        nc.dram_tensor(f"t{l + 1}", [LN[l] * NF], _BF16, kind="ExternalInput")
        for l in range(4)
    ]
    v_d = [nc.dram_tensor(f"v{l + 1}", [LN[l]], _BF16) for l in range(3)]
    o4 = nc.dram_tensor("o4", [LN[3]], _F32, kind="ExternalOutput")

    srcs = [x_d.ap()] + [v.ap() for v in v_d]
    dsts = [v.ap() for v in v_d] + [o4.ap()]
    th_aps = [t.ap() for t in th_d]

    with tile.TileContext(nc) as tc:
        _kernel_body(nc, tc, th_aps, srcs, dsts, LN, LEVEL_PG)
    nc.compile()
    return nc


def _kernel_body(nc, tc, th_aps, srcs, dsts, lns, level_pg) -> None:
    with (
        tc.tile_pool(name="consts", bufs=1) as cp,
        tc.tile_pool(name="th", bufs=3) as thp,
        tc.tile_pool(name="e", bufs=3) as ep,
        tc.tile_pool(name="xs", bufs=3) as xsp,
        tc.tile_pool(name="m", bufs=3) as mp,
        tc.tile_pool(name="pp", bufs=3) as ppp,
        tc.tile_pool(name="ps", bufs=3) as psp,
        tc.tile_pool(name="acc", bufs=3, space="PSUM") as accp,
        tc.tile_pool(name="small", bufs=4) as sp,
    ):
        ident = cp.tile([128, 128], _BF16, tag="id")
        make_identity(nc, ident[:])

        def emit_front(lvl, t, P, G, th_src, xs_src):
            th_t = thp.tile([P, G * NF], _BF16, tag="th")
            nc.sync.dma_start(out=th_t[:], in_=th_src[t])
            xs_t = xsp.tile([P, G * B], _BF16, tag="xs")
            nc.sync.dma_start(out=xs_t[:], in_=xs_src[t])

            e_t = ep.tile([P, G * NF], _BF16, tag="e")
            nc.scalar.activation(e_t[:], th_t[:], mybir.ActivationFunctionType.Exp)
            e3 = e_t[:].rearrange("p (g f) -> p g f", g=G)

            # pair mins, distance-major: m3[:, :, off_d + i] = min(x_i, x_{i+d})
            m_t = mp.tile([P, G * NPAIR], _BF16, tag="m")
            m3 = m_t[:].rearrange("p (g f) -> p g f", g=G)
            x3 = xs_t[:].rearrange("p (g f) -> p g f", g=G)
            for d in range(1, B):
                c = B - d
                o = int(OFFD[d - 1])
                nc.vector.tensor_tensor(
                    m3[:, :, o : o + c],
                    x3[:, :, 0:c],
                    x3[:, :, d:B],
                    op=mybir.AluOpType.min,
                )

            # products (all on DVE: GPSIMD shares the DVE SBUF port, so
            # offloading elementwise work there just serializes)
            ps_t = psp.tile([P, G * B], _BF16, tag="ps")
            nc.vector.tensor_tensor(
                ps_t[:].rearrange("p (g f) -> p g f", g=G),
                e3[:, :, 0:B],
                x3,
                op=mybir.AluOpType.mult,
            )
            pp_t = ppp.tile([P, G * NPAIR], _BF16, tag="pp")
            nc.vector.tensor_tensor(
                pp_t[:].rearrange("p (g f) -> p g f", g=G),
                e3[:, :, B:NF],
                m3,
                op=mybir.AluOpType.mult,
            )

            # PE segmented reduction: accumulate 136 features -> 8 per node
            # via identity-stationary matmuls summing strided j-slices.
            idp = ident[:][0:P, 0:P]
            accn = accp.tile([P, G * 8], _F32, tag="accn")
            accn3 = accn[:].rearrange("p (g j) -> p g j", g=G)
            ps4 = ps_t[:].rearrange("p (g t j) -> p g t j", g=G, t=2, j=8)
            pp4 = pp_t[:].rearrange("p (g t j) -> p g t j", g=G, t=15, j=8)
            for k in range(2):
                nc.tensor.matmul(
                    accn3, idp, ps4[:, :, k, :], start=(k == 0), stop=False
                )
            for k in range(15):
                nc.tensor.matmul(
                    accn3, idp, pp4[:, :, k, :], start=False, stop=(k == 14)
                )

            accd = accp.tile([P, G * 8], _F32, tag="accd")
            accd3 = accd[:].rearrange("p (g j) -> p g j", g=G)
            e4 = e_t[:].rearrange("p (g t j) -> p g t j", g=G, t=17, j=8)
            for k in range(17):
                nc.tensor.matmul(
                    accd3, idp, e4[:, :, k, :], start=(k == 0), stop=(k == 16)
                )
            return accn3, accd3

        def emit_back(ctx):
            accn3, accd3, dst, P, G, out_dt = ctx
            num_t = sp.tile([P, G], _F32, tag="num")
            nc.vector.tensor_reduce(
                num_t[:], accn3, axis=mybir.AxisListType.X, op=mybir.AluOpType.add
            )
            den_t = sp.tile([P, G], _F32, tag="den")
            nc.vector.tensor_reduce(
                den_t[:], accd3, axis=mybir.AxisListType.X, op=mybir.AluOpType.add
            )
            rcp_t = sp.tile([P, G], _F32, tag="rcp")
            nc.vector.reciprocal(rcp_t[:], den_t[:])
            out_t = sp.tile([P, G], out_dt, tag="out")
            nc.vector.tensor_mul(out_t[:], num_t[:], rcp_t[:])
            nc.sync.dma_start(out=dst, in_=out_t[:])

        for lvl in range(len(lns)):
            P, G = level_pg[lvl]
            n_nodes = lns[lvl]
            ntile = P * G
            T = n_nodes // ntile
            assert T * ntile == n_nodes
            th_src = th_aps[lvl].rearrange("(t p f) -> t p f", p=P, f=G * NF)
            xs_src = srcs[lvl].rearrange("(t p q) -> t p q", p=P, q=G * B)
            out_dst = dsts[lvl].rearrange("(t p g) -> t p g", p=P, g=G)
            out_dt = _F32 if lvl == 3 else _BF16

            # software pipelining (lag 2) within a level: tile t's PSUM
            # reads are emitted after tile t+2's front half so the DVE
            # queue never head-of-line blocks on the PE accumulation.
            # Flush at level end: the next level's input DMA must follow
            # this level's output DMAs in the sync-queue FIFO.
            pend = []
            for t in range(T):
                accn3, accd3 = emit_front(lvl, t, P, G, th_src, xs_src)
                pend.append((accn3, accd3, out_dst[t], P, G, out_dt))
                if len(pend) > 2:
                    emit_back(pend.pop(0))
            while pend:
                emit_back(pend.pop(0))


_PROG_CACHE: bass.Bass | None = None
LAST_RESULTS = None  # BassKernelResults of the most recent kernel() call


def _ensure_ntff_hook() -> None:
    """Provide antenv.axon_hooks + the ctypes NTFF hook when the image
    lacks them, so trace=True produces a perfetto profile under axon."""
    import contextlib
    import ctypes
    import sys
    import types

    try:
        from antenv.axon_hooks import get_axon_ntff_profile_hook  # noqa: F401

        return
    except ImportError:
        pass

    import antenv
    import concourse.bass_utils as bu

    holder = {"h": None}
    mod = types.ModuleType("antenv.axon_hooks")
    mod.set_axon_ntff_profile_hook = lambda h: holder.__setitem__("h", h)
    mod.get_axon_ntff_profile_hook = lambda: holder["h"]
    sys.modules["antenv.axon_hooks"] = mod
    antenv.axon_hooks = mod
    bu.upload_artifacts = lambda tmpdir: ""  # no artifact bucket here

    so_path = "/opt/axon/libaxon_pjrt.so"
    try:
        lib = ctypes.CDLL(so_path)
    except OSError:
        return
    if not hasattr(lib, "axon_start_nrt_profile"):
        return
    lib.axon_start_nrt_profile.argtypes = [
        ctypes.POINTER(ctypes.c_int64),
        ctypes.c_size_t,
    ]
    lib.axon_start_nrt_profile.restype = ctypes.c_int64
    lib.axon_stop_nrt_profile.argtypes = [ctypes.c_char_p]
    lib.axon_stop_nrt_profile.restype = ctypes.c_int64

    @contextlib.contextmanager
    def _hook(output_dir, device_ids):
        import jax

        jax.devices()
        if device_ids:
            ids = (ctypes.c_int64 * len(device_ids))(*device_ids)
            rc = lib.axon_start_nrt_profile(ids, len(device_ids))
        else:
            rc = lib.axon_start_nrt_profile(None, 0)
        if rc != 0:
            raise RuntimeError(f"axon_start_nrt_profile rc={rc}")
        try:
            yield
        finally:
            n = lib.axon_stop_nrt_profile(str(output_dir).encode())
            print(f"profile: {n} file(s) written to {output_dir}")

    mod.set_axon_ntff_profile_hook(_hook)


def _choquet_np(vals: np.ndarray, theta: np.ndarray) -> np.ndarray:
    II, JJ = np.triu_indices(B, k=1)
    n = theta.shape[0]
    xs = vals.reshape(n, B).astype(np.float64)
    t = theta.astype(np.float64)
    e = np.exp(t - t.max(axis=1, keepdims=True))
    m = e / e.sum(axis=1, keepdims=True)
    mins = np.minimum(xs[:, II], xs[:, JJ])
    return (m[:, :B] * xs).sum(axis=1) + (m[:, B:] * mins).sum(axis=1)


def kernel(x, theta1, theta2, theta3, theta4, theta5, theta6) -> np.ndarray:
    global _PROG_CACHE, LAST_RESULTS
    import ml_dtypes

    bf16 = ml_dtypes.bfloat16
    x = np.ascontiguousarray(np.asarray(x, dtype=np.float32).reshape(-1)).astype(bf16)
    ths = []
    for th in (theta1, theta2, theta3, theta4):
        th = np.asarray(th, dtype=np.float32)
        ths.append(np.ascontiguousarray(th[:, PERM].astype(bf16)))

    if _PROG_CACHE is None:
        _PROG_CACHE = _build_program()
    nc = _PROG_CACHE

    in_maps = []
    for c in range(NCORE):
        m = {"x": x[c * LEAF_PER_CORE : (c + 1) * LEAF_PER_CORE]}
        for l in range(4):
            rows = LN[l]
            m[f"t{l + 1}"] = ths[l][c * rows : (c + 1) * rows].reshape(-1)
        in_maps.append(m)

    trace = os.environ.get("BASS_KERNEL_TRACE", "0") == "1"
    if trace:
        _ensure_ntff_hook()
    res = run_bass_kernel_spmd(nc, in_maps, list(range(NCORE)), trace=trace)
    LAST_RESULTS = res

    l4 = np.concatenate(
        [np.asarray(res.results[c]["o4"], dtype=np.float32).reshape(-1) for c in range(NCORE)]
    )
    l5 = _choquet_np(l4, np.asarray(theta5, dtype=np.float32))
    l6 = _choquet_np(l5, np.asarray(theta6, dtype=np.float32))
    return l6.astype(np.float32).reshape((1,))
